# revision 14
# baseline (speedup 1.0000x reference)
"""Causal self-attention (B=4, S=2048, D=1024, H=16) on 8 TRN2 NeuronCores.

Sharding: core = (batch b, head-group g) with b = core//2, g = core%2.
Each core computes, for its batch and its 8 heads: QKV projection,
transposed flash-style attention (scores kept k-major so the softmax'd
weights feed the PV matmul directly as the moving operand), and a partial
output projection over its heads' 512 hidden dims.  The host sums the two
partial projections per batch.

Perf notes (v3):
- x is transposed by the DMA crossbar (dma_start_transpose), not the PE.
- Startup streams weights on the scalar hwdge queue and x-transposes on the
  sync queue in parallel, with Q/K/V projection granules riffled so the PE
  starts ~2us in.
- Background PE work is emitted in ~2-matmul granules BETWEEN a group's
  score matmuls and its PV matmuls (the PV waits on exp; granules emitted
  after it would head-of-line block the in-order PE queue).  TRN2 drops the
  PE clock from 2.4 to 1.2 GHz for ~3us after every idle gap, so the PE
  must never go idle.
- exp() extents are clipped on the causal-diagonal chunk pairs.
- Softmax normalization is split into head halves: heads 0-3 are
  normalized while heads 4-7 still run, shortening the dependence tail.
- All output projections are deferred into window 3, which is otherwise
  exp(ACT)-bound.
- Output is stored fp16 (host accumulates partials in fp32).

Numerics: matmul operands are fp16 (1 cycle/row on the PE vs 2 for fp32r,
accumulation still fp32 in PSUM); exp runs with a constant -4 shift, tuned so
fp16 softmax weights neither overflow nor hit the subnormal flush-to-zero
floor (the shift cancels in normalization).
End-to-end relative error ~1e-3 vs the fp32 reference.
"""

import numpy as np

B, S, D, H = 4, 2048, 1024, 16
HS = 64            # head size
NH = 8             # heads per core
C = 512            # per-core q/k/v width (NH * HS)
P = 128
NCORES = 8
DCH = D // P       # 8 contraction chunks for the projections
NW = S // 512      # 4 query windows of 512
KCH = S // P       # 16 key chunks
ESHIFT = -4.0      # exp(0.25*s + ESHIFT); cancels in the softmax ratio

_cache = {}


def _build(has_bias=False):
    key = ("nc", has_bias)
    if key in _cache:
        return _cache[key]

    from contextlib import ExitStack

    import concourse.bass as bass
    import concourse.tile as tile
    from concourse import bacc, mybir
    from concourse.masks import make_upper_triangular

    f32 = mybir.dt.float32
    f16 = mybir.dt.float16
    Exp = mybir.ActivationFunctionType.Exp

    nc = bacc.Bacc(
        "TRN2", target_bir_lowering=False, debug=False, num_devices=NCORES
    )

    x_d = nc.dram_tensor("x", [S, D], f16, kind="ExternalInput").ap()
    wq_d = nc.dram_tensor("wq", [D, C], f16, kind="ExternalInput").ap()
    wk_d = nc.dram_tensor("wk", [D, C], f16, kind="ExternalInput").ap()
    wv_d = nc.dram_tensor("wv", [D, C], f16, kind="ExternalInput").ap()
    wp_d = nc.dram_tensor("wp", [C, D], f16, kind="ExternalInput").ap()
    # per-partition bias columns: col j<4 -> q col-tile j, col 4+j -> k col-tile j
    bqk_d = nc.dram_tensor("bqk", [P, 8], f32, kind="ExternalInput").ap()
    bv_d = nc.dram_tensor("bv", [1, C], f32, kind="ExternalInput").ap()
    bp_d = nc.dram_tensor("bp", [1, D], f32, kind="ExternalInput").ap()
    out_d = nc.dram_tensor("out", [S, D], f16, kind="ExternalOutput").ap()

    with tile.TileContext(nc) as tc, ExitStack() as ctx:
        ctx.enter_context(nc.allow_low_precision(reason="fp16 attention"))

        const = ctx.enter_context(tc.tile_pool(name="const", bufs=1))
        persist = ctx.enter_context(tc.tile_pool(name="persist", bufs=1))
        xtp = ctx.enter_context(tc.tile_pool(name="xtp", bufs=4))
        qtw = ctx.enter_context(tc.tile_pool(name="qtw", bufs=2))
        otw = ctx.enter_context(tc.tile_pool(name="otw", bufs=4))
        expool = ctx.enter_context(tc.tile_pool(name="expool", bufs=4))
        denpool = ctx.enter_context(tc.tile_pool(name="denpool", bufs=3))
        rhpool = ctx.enter_context(tc.tile_pool(name="rhpool", bufs=4))
        stpool = ctx.enter_context(tc.tile_pool(name="stpool", bufs=3))

        pS = ctx.enter_context(tc.tile_pool(name="pS", bufs=2, space="PSUM"))
        pPV = ctx.enter_context(tc.tile_pool(name="pPV", bufs=1, space="PSUM"))
        pDEN = ctx.enter_context(tc.tile_pool(name="pDEN", bufs=1, space="PSUM"))
        pMISC = ctx.enter_context(tc.tile_pool(name="pMISC", bufs=1, space="PSUM"))
        pAUX = ctx.enter_context(tc.tile_pool(name="pAUX", bufs=1, space="PSUM"))

        tri = const.tile([P, P], f32, tag="tri")
        make_upper_triangular(nc, tri, val=1.0, diag=True)  # tri[k,q]=1 iff q>=k
        eshift_sb = const.tile([P, 1], f32, tag="eshift")
        nc.vector.memset(eshift_sb, ESHIFT)
        ones8 = const.tile([P, 8], f32, tag="ones8")
        nc.vector.memset(ones8, 1.0)
        ohf = const.tile([1, 64], f32, tag="ohf")
        nc.vector.memset(ohf, 0.0)
        for h in range(NH):
            nc.vector.memset(ohf[0:1, h * 9:h * 9 + 1], 1.0)
        onehot = const.tile([1, NH, NH], f16, tag="onehot")
        nc.vector.tensor_copy(onehot, ohf[:].rearrange("p (a b) -> p a b", b=NH))
        # band-select tiles: sel_a rows 0-3 = heads 0-3 bands over cols 0..255
        # (col-tiles 0,1), sel_b rows 0-3 = heads 4-7 over cols 0..255 (2,3)
        bandf = const.tile([4, 256], f32, tag="bandf")
        nc.gpsimd.memset(bandf, 1.0)
        nc.gpsimd.affine_select(
            out=bandf, in_=bandf, compare_op=mybir.AluOpType.is_ge,
            fill=0.0, base=0, pattern=[[1, 256]], channel_multiplier=-64)
        nc.gpsimd.affine_select(
            out=bandf, in_=bandf, compare_op=mybir.AluOpType.is_ge,
            fill=0.0, base=63, pattern=[[-1, 256]], channel_multiplier=64)
        sel_ab = const.tile([4, 256], f16, tag="sel_ab")
        nc.vector.tensor_copy(sel_ab, bandf[:])

        if has_bias:
            bqk_sb = const.tile([P, 8], f32, tag="bqk")
            nc.sync.dma_start(bqk_sb, bqk_d)
            bv_bc = const.tile([P, C], f32, tag="bv_bc")
            nc.sync.dma_start(
                bv_bc,
                bass.AP(tensor=bv_d.tensor, offset=bv_d.offset,
                        ap=[[0, P], list(bv_d.ap[-1])]),
            )
            bp_bc = const.tile([P, D], f32, tag="bp_bc")
            nc.sync.dma_start(
                bp_bc,
                bass.AP(tensor=bp_d.tensor, offset=bp_d.offset,
                        ap=[[0, P], list(bp_d.ap[-1])]),
            )

        wq_sb = persist.tile([P, DCH, C], f16, tag="wq")
        wk_sb = persist.tile([P, DCH, C], f16, tag="wk")
        wv_sb = persist.tile([P, DCH, C], f16, tag="wv")
        wp_sb = persist.tile([P, 4, D], f16, tag="wp")
        KT = persist.tile([P, 4, S], f16, tag="KT")
        Vt = persist.tile([P, KCH, NH * 65], f16, tag="Vt")

        # ---------- unit generators (emitted lazily for interleaving) ----------

        def xT_dma(w, xT, eng=None, dhs=range(DCH)):
            """DMA-transpose x rows of window w into xT [P, DCH, 512].
            xT[p, dh, s] = x[w*512+s, dh*128+p].  Crossbar tiles are 16x128;
            each per-dh transfer is 32 tiles (~1.3us on the queue)."""
            eng = eng or nc.sync
            for dh in dhs:
                eng.dma_start_transpose(
                    xT[:, dh, :], x_d[w * 512:(w + 1) * 512, dh * P:(dh + 1) * P])

        def qk_granules(w, xT, qt, pools=None, order=None):
            """Q^T / K^T projections for window w from xT, in 2-matmul granules.
            Order: ct-major with q before k so head 2ct/2ct+1's inputs complete
            as early as possible."""
            pools = pools or [pAUX]
            units = order or [(ct, qk) for ct in range(4) for qk in range(2)]
            for ui, (ct, qk) in enumerate(units):
                ps = [None]
                pool = pools[ui % len(pools)]
                for gd in range(4):
                    def unit(w=w, ct=ct, qk=qk, gd=gd, xT=xT, qt=qt, ps=ps,
                             pool=pool):
                        if gd == 0:
                            ps[0] = pool.tile([P, 512], f32, name="aux_ps",
                                              tag="aux" if pool is pAUX
                                              else "misc")
                        wsb = wq_sb if qk == 0 else wk_sb
                        for d in (2 * gd, 2 * gd + 1):
                            nc.tensor.matmul(
                                ps[0], wsb[:, d, ct * P:(ct + 1) * P],
                                xT[:, d, :],
                                start=(d == 0), stop=(d == DCH - 1))
                        if gd == 3:
                            dest = (qt[:, ct, :] if qk == 0
                                    else KT[:, ct, w * 512:(w + 1) * 512])
                            if has_bias:
                                nc.vector.tensor_scalar_add(
                                    dest, ps[0],
                                    bqk_sb[:, qk * 4 + ct:qk * 4 + ct + 1])
                            else:
                                nc.vector.tensor_copy(dest, ps[0])
                    yield unit

        def v_granules(w, xT, pools=None):
            """V for the 4 key chunks of window w, head-grouped with ones col."""
            pools = pools or [pAUX]
            for st in range(4):
                ps = [None]
                pool = pools[st % len(pools)]
                for gd in range(4):
                    def unit(w=w, st=st, gd=gd, xT=xT, ps=ps, pool=pool):
                        if gd == 0:
                            ps[0] = pool.tile([P, 512], f32, name="aux_ps",
                                              tag="aux" if pool is pAUX
                                              else "misc")
                        for d in (2 * gd, 2 * gd + 1):
                            nc.tensor.matmul(ps[0],
                                             xT[:, d, st * P:(st + 1) * P],
                                             wv_sb[:, d, :],
                                             start=(d == 0), stop=(d == DCH - 1))
                        if gd == 3:
                            kc = 4 * w + st
                            vtv = Vt[:, kc, :].rearrange("p (h c) -> p h c", c=65)
                            if has_bias:
                                nc.vector.tensor_add(
                                    vtv[:, :, 0:64],
                                    ps[0].rearrange("p (h c) -> p h c", c=64),
                                    bv_bc[:].rearrange("p (h c) -> p h c", c=64))
                            else:
                                nc.vector.tensor_copy(
                                    vtv[:, :, 0:64],
                                    ps[0].rearrange("p (h c) -> p h c", c=64))
                            nc.vector.tensor_copy(vtv[:, :, 64:65],
                                                  ones8[:, :].unsqueeze(2))
                    yield unit

        def proj_granules(w, ot, pools=None):
            """Output projection for window w's 512 seq rows (partial over C),
            2-matmul granules; the closing granule stages and DMAs out fp16."""
            pools = pools or [pMISC]
            for ui in range(8):
                st, cw = ui // 2, ui % 2
                pp = [None]
                pool = pools[ui % len(pools)]
                for gj in range(2):
                    def unit(w=w, st=st, cw=cw, gj=gj, ot=ot, pp=pp, pool=pool):
                        if gj == 0:
                            pp[0] = pool.tile([P, 512], f32, name="proj_pp",
                                              tag="aux" if pool is pAUX
                                              else ("pv" if pool is pPV
                                                    else "misc"))
                        for j in (2 * gj, 2 * gj + 1):
                            nc.tensor.matmul(
                                pp[0], ot[:, j, st * P:(st + 1) * P],
                                wp_sb[:, j, cw * 512:(cw + 1) * 512],
                                start=(j == 0), stop=(j == 3))
                        if gj == 1:
                            stg = stpool.tile([P, 512], f16, tag="stg")
                            if has_bias:
                                nc.vector.tensor_add(
                                    stg, pp[0],
                                    bp_bc[:, cw * 512:(cw + 1) * 512])
                            else:
                                nc.vector.tensor_copy(stg, pp[0])
                            row0 = (4 * w + st) * P
                            nc.sync.dma_start(
                                out_d[row0:row0 + P,
                                      cw * 512:(cw + 1) * 512], stg)
                    yield unit

        # debt-based background filler -------------------------------------
        GRAN_NS = 430.0          # ~2 fp16 matmuls of 512 rows at 2.4 GHz
        ACT_OH = 262.0           # measured ACT fixed overhead per instruction
        state = {"debt": 0.0, "bg": []}

        def fill(act_ns, pe_ns):
            """Inject background granules to cover PE idle while the scalar
            engine runs exp (or a cross-engine dependency settles)."""
            state["debt"] += act_ns - pe_ns
            while state["debt"] > 0.5 * GRAN_NS and state["bg"]:
                state["bg"].pop(0)()
                state["debt"] -= GRAN_NS
            if state["debt"] < -3 * GRAN_NS:
                state["debt"] = -3 * GRAN_NS

        def attention_head(w, h, qt, ot, den_ps_ab):
            """Scores+exp+PV for one head; unnormalized O -> ot, denominator
            scattered into row h%4 of den_ps_ab[h//4] via a one-hot matmul."""
            ct, po = h // 2, (h % 2) * 64
            pv = pPV.tile([65, 512], f32, tag="pv")
            last_kc = 4 * w + 3
            for g2 in range(2 * (w + 1)):      # 2-chunk half groups
                sc = pS.tile([P, 2, 512], f32, tag="sc")
                pe_ns = 0.0
                for rr in range(2):
                    kc = 2 * g2 + rr
                    # masked q-range is never read downstream; keep the very
                    # first head full-extent so the psum slots are initialized
                    s0 = max(0, kc - 4 * w) * P if (w, h) != (0, 0) else 0
                    nc.tensor.matmul(
                        sc[:, rr, s0:],
                        KT[po:po + 64, ct, kc * P:(kc + 1) * P],
                        qt[po:po + 64, ct, s0:],
                        start=True, stop=True)
                    pe_ns += (512 - s0) * 0.4166
                # clip exp on the upper diagonal pair: chunks (4w+2, 4w+3)
                # only feed queries >= 256 (their tri-masked PV extents)
                e0 = 0
                if (w, h) != (0, 0) and 2 * g2 == 4 * w + 2:
                    e0 = 256
                ex = expool.tile([P, 2, 512], f16, tag="ex")
                nc.scalar.activation(ex[:, :, e0:], sc[:, :, e0:], Exp,
                                     scale=0.25, bias=eshift_sb[:])
                act_ns = (1024 - 2 * e0) * 0.853 + ACT_OH
                # background fill BEFORE the PV matmuls: PV waits on exp, and
                # anything emitted after it would head-of-line block the PE
                fill(act_ns, pe_ns)
                for rr in range(2):
                    kc = 2 * g2 + rr
                    r = kc - 4 * w            # >=0 only inside the diag block
                    q0 = 0
                    if r >= 0:
                        # only q >= r*P can attend this chunk; clip the PV
                        # extent instead of zero-filling the masked region
                        q0 = r * P
                        nc.vector.tensor_mul(ex[:, rr, q0:q0 + P],
                                             ex[:, rr, q0:q0 + P], tri[:])
                    nc.tensor.matmul(pv[:, q0:], Vt[:, kc, h * 65:(h + 1) * 65],
                                     ex[:, rr, q0:],
                                     start=(kc == 0), stop=(kc == last_kc))
            # stash unnormalized O scaled by 1/4 (fp16 range headroom);
            # scatter the denominator into row h%4 of this half's den psum
            nc.vector.tensor_scalar_mul(ot[po:po + 64, ct, :], pv[0:64, :], 0.25)
            den_h = denpool.tile([1, 512], f16, tag="den")
            nc.vector.tensor_copy(den_h, pv[64:65, :])
            # cover the DVE copy latency before the PE scatter consumes it
            fill(700.0, 0.0)
            den_ps = den_ps_ab[h // 4]
            hh = h % 4
            oh = onehot[:, h, 0:4] if h < 4 else onehot[:, h, 4:8]
            nc.tensor.matmul(den_ps, oh, den_h[:],
                             start=(hh == 0), stop=(hh == 3))
            fill(0.0, 512 * 0.4166)

        def norm_units(w, half, ot, den_ps, bc_pools=None):
            """Softmax normalization of ot columns 2*half..2*half+1 (heads
            4*half..4*half+3), reading this half's denominator psum."""
            bc_pools = bc_pools or [pMISC]
            shared = {}

            def recip_unit(ot=ot, den_ps=den_ps):
                rall = rhpool.tile([4, 512], f32, tag="rall")
                nc.vector.reciprocal_approx_fast(rall, den_ps[:])
                rh = rhpool.tile([4, 512], f16, tag="rh")
                # x4 so small reciprocals clear the fp16 subnormal floor; the
                # 4*(1/4) pair cancels in the final normalize multiply
                nc.vector.tensor_scalar_mul(rh, rall, 4.0)
                shared["rh"] = rh
            yield recip_unit
            for jj in range(2):
                j = 2 * half + jj
                pool = bc_pools[jj % len(bc_pools)]
                def unit(j=j, jj=jj, ot=ot, w=w, pool=pool):
                    rh = shared["rh"]
                    # sel_ab[:, 128jj..] row m selects head 4*half + 2jj+(m>=64)
                    bcp = pool.tile([P, 512], f32, name="bc_ps",
                                    tag="aux" if pool is pAUX
                                    else ("pv" if pool is pPV else "misc"))
                    nc.tensor.matmul(bcp, sel_ab[:, jj * P:(jj + 1) * P],
                                     rh[:], start=True, stop=True)
                    # multiply reads the broadcast factors straight from PSUM
                    nc.vector.tensor_mul(ot[:, j, :], ot[:, j, :], bcp[:])
                yield unit

        # ---------------------------- schedule ----------------------------

        xT_all = [xtp.tile([P, DCH, 512], f16, tag="xT", name="xT")
                  for _ in range(NW)]
        qt_cur = qtw.tile([P, 4, 512], f16, tag="qt")
        # startup: weights on the scalar hwdge queue, x-transposes on sync,
        # so both streams land in parallel; riffle the first window's
        # projection granules so the PE starts as soon as chunk 0 arrives
        for d in range(DCH):
            nc.scalar.dma_start(wq_sb[:, d, :], wq_d[d * P:(d + 1) * P, :])
            nc.scalar.dma_start(wk_sb[:, d, :], wk_d[d * P:(d + 1) * P, :])
        xT_dma(0, xT_all[0], eng=nc.sync)
        for d in range(DCH):
            nc.scalar.dma_start(wv_sb[:, d, :], wv_d[d * P:(d + 1) * P, :])
        xT_dma(1, xT_all[1], eng=nc.sync)
        for j in range(4):            # proj weights are not needed until w3
            nc.scalar.dma_start(wp_sb[:, j, :], wp_d[j * P:(j + 1) * P, :])
        xT_dma(2, xT_all[2], eng=nc.sync)
        xT_dma(3, xT_all[3], eng=nc.sync)

        # qk granules pace themselves against the weight-chunk arrivals on
        # the scalar queue; v granules follow (wv lands after wq/wk anyway).
        # Separate psum pools so the two unit streams never alias mid-flight.
        for u in qk_granules(0, xT_all[0], qt_cur, pools=[pAUX]):
            u()
        for u in v_granules(0, xT_all[0], pools=[pMISC]):
            u()

        pending = []            # norm units that must run after window w
        deferred_proj = []      # proj granules, all pushed into window 3
        for w in range(NW):
            ot_cur = otw.tile([P, 4, 512], f16, tag="ot")
            den_a = pDEN.tile([4, 512], f32, tag="den_ps", name="den_a")
            den_b = pDEN.tile([4, 512], f32, tag="den_ps", name="den_b")

            bg = list(pending)
            pending = []
            if w == NW - 1:
                bg += deferred_proj
                deferred_proj = []
            if w + 1 < NW:
                qt_next = qtw.tile([P, 4, 512], f16, tag="qt")
                bg += list(qk_granules(w + 1, xT_all[w + 1], qt_next))
                bg += list(v_granules(w + 1, xT_all[w + 1], pools=[pAUX]))
            else:
                qt_next = None
            state["bg"] = bg
            state["debt"] = 0.0

            norm_a = list(norm_units(w, 0, ot_cur, den_a))
            for h in range(NH):
                attention_head(w, h, qt_cur, ot_cur, (den_a, den_b))
                if h == 3:
                    # run the half-a reciprocal now (DVE-only, frees den_a
                    # before h4's scatter rotates the den psum); the bc+mul
                    # units become background work for the second half
                    norm_a[0]()
                    state["bg"] = norm_a[1:] + state["bg"]
            for u in state["bg"]:
                u()
            state["bg"] = []

            norm_b = list(norm_units(w, 1, ot_cur, den_b,
                                     bc_pools=[pMISC, pAUX]
                                     if w == NW - 1 else None))
            if w + 1 < NW:
                norm_b[0]()          # reciprocal frees den_b psum early
                pending = norm_b[1:]
                # deferred projections run inside window 3, where pAUX is
                # otherwise idle (norm/bc stay on pMISC, so the norm-half-a
                # units can be inserted mid-unit without psum aliasing)
                deferred_proj += list(proj_granules(w, ot_cur, pools=[pAUX]))
            else:
                for u in norm_b:
                    u()
                for u in proj_granules(w, ot_cur,
                                       pools=[pMISC, pAUX, pPV]):
                    u()

            qt_cur = qt_next

    nc.compile()
    _cache[key] = nc
    return nc


def _make_in_maps(input_data, w_qkv, b_qkv, w_proj, b_proj):
    x = np.asarray(input_data, dtype=np.float32).astype(np.float16)
    wqkv = np.asarray(w_qkv, dtype=np.float32).astype(np.float16)
    bqkv = np.asarray(b_qkv, dtype=np.float32)
    wp = np.asarray(w_proj, dtype=np.float32).astype(np.float16)
    bp = np.asarray(b_proj, dtype=np.float32)

    in_maps = []
    for core in range(NCORES):
        b, g = core // 2, core % 2
        cs = slice(g * C, (g + 1) * C)
        bq = bqkv[0 * D:1 * D][cs]
        bk = bqkv[1 * D:2 * D][cs]
        bqk = np.empty((P, 8), np.float32)
        for j in range(4):
            bqk[:, j] = bq[j * P:(j + 1) * P]
            bqk[:, 4 + j] = bk[j * P:(j + 1) * P]
        in_maps.append({
            "x": np.ascontiguousarray(x[b]),
            "wq": np.ascontiguousarray(wqkv[:, 0 * D:1 * D][:, cs]),
            "wk": np.ascontiguousarray(wqkv[:, 1 * D:2 * D][:, cs]),
            "wv": np.ascontiguousarray(wqkv[:, 2 * D:3 * D][:, cs]),
            "wp": np.ascontiguousarray(wp[cs, :]),
            "bqk": bqk,
            "bv": np.ascontiguousarray(bqkv[2 * D:3 * D][cs]).reshape(1, C),
            "bp": (bp if g == 0 else np.zeros_like(bp)).reshape(1, D),
        })
    return in_maps


def kernel(input_data, w_qkv, b_qkv, w_proj, b_proj):
    from concourse.bass_utils import run_bass_kernel_spmd

    has_bias = bool(np.any(np.asarray(b_qkv)) or np.any(np.asarray(b_proj)))
    nc = _build(has_bias)
    in_maps = _make_in_maps(input_data, w_qkv, b_qkv, w_proj, b_proj)
    res = run_bass_kernel_spmd(nc, in_maps, core_ids=list(range(NCORES)))
    parts = [np.asarray(res.results[i]["out"], dtype=np.float32)
             for i in range(NCORES)]
    out = np.stack([parts[2 * b] + parts[2 * b + 1] for b in range(B)])
    return out.astype(np.float32)


# revision 22
# speedup vs baseline: 1.3288x; 1.3288x over previous
"""Causal self-attention (B=4, S=2048, D=1024, H=16) on 8 TRN2 NeuronCores.

Sharding: core = (batch b, head-group g) with b = core//2, g = core%2.
Each core computes, for its batch and its 8 heads: QKV projection,
transposed flash-style attention (scores kept k-major so the softmax'd
weights feed the PV matmul directly as the moving operand), and a partial
output projection over its heads' 512 hidden dims.  The host sums the two
partial projections per batch.

Perf notes (v4):
- x is transposed on the PE in d-major units (all four seq-tiles of one
  dh-pair per unit), so the first Q-projection granule's moving operand is
  complete after one unit (~0.5us of PE).  DMA-crossbar transposes were
  tried and are ~10x slower than the cost model claims on this hardware.
- Startup streams x on both hwdge queues and weights on the scalar queue
  in parallel; transpose/projection units rotate across the idle attention
  psum pools so nothing serializes on a single psum bank.
- Background PE work is emitted in ~2-matmul granules BETWEEN a group's
  score matmuls and its PV matmuls (the PV waits on exp; granules emitted
  after it would head-of-line block the in-order PE queue).  TRN2 drops the
  PE clock from 2.4 to 1.2 GHz for ~3us after every idle gap, so the PE
  must never go idle.
- exp() extents are clipped on the causal-diagonal chunk pairs.
- Softmax normalization is split into head halves: heads 0-3 are
  normalized while heads 4-7 still run, shortening the dependence tail.
- All output projections are deferred into window 3, which is otherwise
  exp(ACT)-bound.
- Output is stored fp16 (host accumulates partials in fp32).

Numerics: matmul operands are fp16 (1 cycle/row on the PE vs 2 for fp32r,
accumulation still fp32 in PSUM); exp runs with a constant -4 shift, tuned so
fp16 softmax weights neither overflow nor hit the subnormal flush-to-zero
floor (the shift cancels in normalization).
End-to-end relative error ~1e-3 vs the fp32 reference.
"""

import numpy as np

B, S, D, H = 4, 2048, 1024, 16
HS = 64            # head size
NH = 8             # heads per core
C = 512            # per-core q/k/v width (NH * HS)
P = 128
NCORES = 8
DCH = D // P       # 8 contraction chunks for the projections
NW = S // 512      # 4 query windows of 512
KCH = S // P       # 16 key chunks
ESHIFT = -4.0      # exp(0.25*s + ESHIFT); cancels in the softmax ratio

_cache = {}


def _build(has_bias=False):
    key = ("nc", has_bias)
    if key in _cache:
        return _cache[key]

    from contextlib import ExitStack

    import concourse.bass as bass
    import concourse.tile as tile
    from concourse import bacc, mybir
    from concourse.masks import make_identity, make_upper_triangular

    f32 = mybir.dt.float32
    f16 = mybir.dt.float16
    Exp = mybir.ActivationFunctionType.Exp

    nc = bacc.Bacc(
        "TRN2", target_bir_lowering=False, debug=False, num_devices=NCORES
    )

    x_d = nc.dram_tensor("x", [S, D], f16, kind="ExternalInput").ap()
    wq_d = nc.dram_tensor("wq", [D, C], f16, kind="ExternalInput").ap()
    wk_d = nc.dram_tensor("wk", [D, C], f16, kind="ExternalInput").ap()
    wv_d = nc.dram_tensor("wv", [D, C], f16, kind="ExternalInput").ap()
    wp_d = nc.dram_tensor("wp", [C, D], f16, kind="ExternalInput").ap()
    # per-partition bias columns: col j<4 -> q col-tile j, col 4+j -> k col-tile j
    bqk_d = nc.dram_tensor("bqk", [P, 8], f32, kind="ExternalInput").ap()
    bv_d = nc.dram_tensor("bv", [1, C], f32, kind="ExternalInput").ap()
    bp_d = nc.dram_tensor("bp", [1, D], f32, kind="ExternalInput").ap()
    out_d = nc.dram_tensor("out", [S, D], f16, kind="ExternalOutput").ap()

    with tile.TileContext(nc) as tc, ExitStack() as ctx:
        ctx.enter_context(nc.allow_low_precision(reason="fp16 attention"))

        const = ctx.enter_context(tc.tile_pool(name="const", bufs=1))
        persist = ctx.enter_context(tc.tile_pool(name="persist", bufs=1))
        xload = ctx.enter_context(tc.tile_pool(name="xload", bufs=8))
        xtp = ctx.enter_context(tc.tile_pool(name="xtp", bufs=2))
        qtw = ctx.enter_context(tc.tile_pool(name="qtw", bufs=2))
        otw = ctx.enter_context(tc.tile_pool(name="otw", bufs=4))
        expool = ctx.enter_context(tc.tile_pool(name="expool", bufs=4))
        denpool = ctx.enter_context(tc.tile_pool(name="denpool", bufs=3))
        rhpool = ctx.enter_context(tc.tile_pool(name="rhpool", bufs=4))
        stpool = ctx.enter_context(tc.tile_pool(name="stpool", bufs=3))

        pS = ctx.enter_context(tc.tile_pool(name="pS", bufs=2, space="PSUM"))
        pPV = ctx.enter_context(tc.tile_pool(name="pPV", bufs=1, space="PSUM"))
        pDEN = ctx.enter_context(tc.tile_pool(name="pDEN", bufs=1, space="PSUM"))
        pMISC = ctx.enter_context(tc.tile_pool(name="pMISC", bufs=1, space="PSUM"))
        pAUX = ctx.enter_context(tc.tile_pool(name="pAUX", bufs=1, space="PSUM"))

        identf = const.tile([P, P], f32, tag="identf")
        make_identity(nc, identf)
        ident = const.tile([P, P], f16, tag="ident")
        nc.vector.tensor_copy(ident, identf)
        tri = const.tile([P, P], f32, tag="tri")
        make_upper_triangular(nc, tri, val=1.0, diag=True)  # tri[k,q]=1 iff q>=k
        eshift_sb = const.tile([P, 1], f32, tag="eshift")
        nc.vector.memset(eshift_sb, ESHIFT)
        ones8 = const.tile([P, 8], f32, tag="ones8")
        nc.vector.memset(ones8, 1.0)
        ohf = const.tile([1, 64], f32, tag="ohf")
        nc.vector.memset(ohf, 0.0)
        for h in range(NH):
            nc.vector.memset(ohf[0:1, h * 9:h * 9 + 1], 1.0)
        onehot = const.tile([1, NH, NH], f16, tag="onehot")
        nc.vector.tensor_copy(onehot, ohf[:].rearrange("p (a b) -> p a b", b=NH))
        # band-select tiles: sel_a rows 0-3 = heads 0-3 bands over cols 0..255
        # (col-tiles 0,1), sel_b rows 0-3 = heads 4-7 over cols 0..255 (2,3)
        bandf = const.tile([4, 256], f32, tag="bandf")
        nc.gpsimd.memset(bandf, 1.0)
        nc.gpsimd.affine_select(
            out=bandf, in_=bandf, compare_op=mybir.AluOpType.is_ge,
            fill=0.0, base=0, pattern=[[1, 256]], channel_multiplier=-64)
        nc.gpsimd.affine_select(
            out=bandf, in_=bandf, compare_op=mybir.AluOpType.is_ge,
            fill=0.0, base=63, pattern=[[-1, 256]], channel_multiplier=64)
        sel_ab = const.tile([4, 256], f16, tag="sel_ab")
        nc.vector.tensor_copy(sel_ab, bandf[:])

        if has_bias:
            bqk_sb = const.tile([P, 8], f32, tag="bqk")
            nc.sync.dma_start(bqk_sb, bqk_d)
            bv_bc = const.tile([P, C], f32, tag="bv_bc")
            nc.sync.dma_start(
                bv_bc,
                bass.AP(tensor=bv_d.tensor, offset=bv_d.offset,
                        ap=[[0, P], list(bv_d.ap[-1])]),
            )
            bp_bc = const.tile([P, D], f32, tag="bp_bc")
            nc.sync.dma_start(
                bp_bc,
                bass.AP(tensor=bp_d.tensor, offset=bp_d.offset,
                        ap=[[0, P], list(bp_d.ap[-1])]),
            )

        wq_sb = persist.tile([P, DCH, C], f16, tag="wq")
        wk_sb = persist.tile([P, DCH, C], f16, tag="wk")
        wv_sb = persist.tile([P, DCH, C], f16, tag="wv")
        wp_sb = persist.tile([P, 4, D], f16, tag="wp")
        KT = persist.tile([P, 4, S], f16, tag="KT")
        Vt = persist.tile([P, KCH, NH * 65], f16, tag="Vt")

        # ---------- unit generators (emitted lazily for interleaving) ----------

        def x_dma(w, xts, eng=None):
            """Plain DMA of window w's x rows into 4 seq-tiles [P, D]."""
            eng = eng or nc.sync
            for st in range(4):
                row0 = (4 * w + st) * P
                eng.dma_start(xts[st], x_d[row0:row0 + P, :])

        def xT_granules(w, xts, xT, pools=None):
            """PE-transpose window w's x into xT [P, DCH, 512], one dh-PAIR
            per granule (all 4 seq-tiles), so consumers needing low dh chunks
            unblock after a single granule.  psum slot order (dhh, st)."""
            pools = pools or [pAUX]
            for jd in range(4):
                pool = pools[jd % len(pools)]
                def unit(w=w, jd=jd, xts=xts, xT=xT, pool=pool):
                    ptr = pool.tile([P, 2, 4, P], f16, name="tr_ps",
                                    tag="aux" if pool is pAUX
                                    else ("pv" if pool is pPV
                                          else ("den_ps" if pool is pDEN
                                                else "misc")))
                    for dhh in range(2):
                        d = 2 * jd + dhh
                        for st in range(4):
                            nc.tensor.transpose(
                                ptr[:, dhh, st, :],
                                xts[st][:, d * P:(d + 1) * P], ident[:])
                    nc.vector.tensor_copy(
                        xT[:, 2 * jd:2 * jd + 2, :]
                        .rearrange("p a (b c) -> p a b c", c=P), ptr[:])
                yield unit

        def qk_granules(w, xT, qt, pools=None, order=None):
            """Q^T / K^T projections for window w from xT, in 2-matmul granules.
            Order: ct-major with q before k so head 2ct/2ct+1's inputs complete
            as early as possible."""
            pools = pools or [pAUX]
            units = order or [(ct, qk) for ct in range(4) for qk in range(2)]
            for ui, (ct, qk) in enumerate(units):
                ps = [None]
                pool = pools[ui % len(pools)]
                for gd in range(4):
                    def unit(w=w, ct=ct, qk=qk, gd=gd, xT=xT, qt=qt, ps=ps,
                             pool=pool):
                        if gd == 0:
                            ps[0] = pool.tile([P, 512], f32, name="aux_ps",
                                              tag="aux" if pool is pAUX
                                              else "misc")
                        wsb = wq_sb if qk == 0 else wk_sb
                        for d in (2 * gd, 2 * gd + 1):
                            nc.tensor.matmul(
                                ps[0], wsb[:, d, ct * P:(ct + 1) * P],
                                xT[:, d, :],
                                start=(d == 0), stop=(d == DCH - 1))
                        if gd == 3:
                            dest = (qt[:, ct, :] if qk == 0
                                    else KT[:, ct, w * 512:(w + 1) * 512])
                            if has_bias:
                                nc.vector.tensor_scalar_add(
                                    dest, ps[0],
                                    bqk_sb[:, qk * 4 + ct:qk * 4 + ct + 1])
                            else:
                                nc.vector.tensor_copy(dest, ps[0])
                    yield unit

        def v_granules(w, xT, pools=None):
            """V for the 4 key chunks of window w, head-grouped with ones col."""
            pools = pools or [pAUX]
            for st in range(4):
                ps = [None]
                pool = pools[st % len(pools)]
                for gd in range(4):
                    def unit(w=w, st=st, gd=gd, xT=xT, ps=ps, pool=pool):
                        if gd == 0:
                            ps[0] = pool.tile([P, 512], f32, name="aux_ps",
                                              tag="aux" if pool is pAUX
                                              else "misc")
                        for d in (2 * gd, 2 * gd + 1):
                            nc.tensor.matmul(ps[0],
                                             xT[:, d, st * P:(st + 1) * P],
                                             wv_sb[:, d, :],
                                             start=(d == 0), stop=(d == DCH - 1))
                        if gd == 3:
                            kc = 4 * w + st
                            vtv = Vt[:, kc, :].rearrange("p (h c) -> p h c", c=65)
                            if has_bias:
                                nc.vector.tensor_add(
                                    vtv[:, :, 0:64],
                                    ps[0].rearrange("p (h c) -> p h c", c=64),
                                    bv_bc[:].rearrange("p (h c) -> p h c", c=64))
                            else:
                                nc.vector.tensor_copy(
                                    vtv[:, :, 0:64],
                                    ps[0].rearrange("p (h c) -> p h c", c=64))
                            nc.vector.tensor_copy(vtv[:, :, 64:65],
                                                  ones8[:, :].unsqueeze(2))
                    yield unit

        def proj_granules(w, ot, pools=None):
            """Output projection for window w's 512 seq rows (partial over C),
            2-matmul granules; the closing granule stages and DMAs out fp16."""
            pools = pools or [pMISC]
            for ui in range(8):
                st, cw = ui // 2, ui % 2
                pp = [None]
                pool = pools[ui % len(pools)]
                for gj in range(2):
                    def unit(w=w, st=st, cw=cw, gj=gj, ot=ot, pp=pp, pool=pool):
                        if gj == 0:
                            pp[0] = pool.tile([P, 512], f32, name="proj_pp",
                                              tag="aux" if pool is pAUX
                                              else ("pv" if pool is pPV
                                                    else "misc"))
                        for j in (2 * gj, 2 * gj + 1):
                            nc.tensor.matmul(
                                pp[0], ot[:, j, st * P:(st + 1) * P],
                                wp_sb[:, j, cw * 512:(cw + 1) * 512],
                                start=(j == 0), stop=(j == 3))
                        if gj == 1:
                            stg = stpool.tile([P, 512], f16, tag="stg")
                            if has_bias:
                                nc.vector.tensor_add(
                                    stg, pp[0],
                                    bp_bc[:, cw * 512:(cw + 1) * 512])
                            else:
                                nc.vector.tensor_copy(stg, pp[0])
                            row0 = (4 * w + st) * P
                            nc.sync.dma_start(
                                out_d[row0:row0 + P,
                                      cw * 512:(cw + 1) * 512], stg)
                    yield unit

        # debt-based background filler -------------------------------------
        GRAN_NS = 430.0          # ~2 fp16 matmuls of 512 rows at 2.4 GHz
        ACT_OH = 262.0           # measured ACT fixed overhead per instruction
        state = {"debt": 0.0, "bg": []}

        def fill(act_ns, pe_ns):
            """Inject background granules to cover PE idle while the scalar
            engine runs exp (or a cross-engine dependency settles)."""
            state["debt"] += act_ns - pe_ns
            while state["debt"] > 0.5 * GRAN_NS and state["bg"]:
                state["bg"].pop(0)()
                state["debt"] -= GRAN_NS
            if state["debt"] < -3 * GRAN_NS:
                state["debt"] = -3 * GRAN_NS

        def attention_head(w, h, qt, ot, den_ps_ab):
            """Scores+exp+PV for one head; unnormalized O -> ot, denominator
            scattered into row h%4 of den_ps_ab[h//4] via a one-hot matmul."""
            ct, po = h // 2, (h % 2) * 64
            pv = pPV.tile([65, 512], f32, tag="pv")
            last_kc = 4 * w + 3
            for g2 in range(2 * (w + 1)):      # 2-chunk half groups
                sc = pS.tile([P, 2, 512], f32, tag="sc")
                pe_ns = 0.0
                for rr in range(2):
                    kc = 2 * g2 + rr
                    # masked q-range is never read downstream; keep the very
                    # first head full-extent so the psum slots are initialized
                    s0 = max(0, kc - 4 * w) * P if (w, h) != (0, 0) else 0
                    nc.tensor.matmul(
                        sc[:, rr, s0:],
                        KT[po:po + 64, ct, kc * P:(kc + 1) * P],
                        qt[po:po + 64, ct, s0:],
                        start=True, stop=True)
                    pe_ns += (512 - s0) * 0.4166
                # clip exp on the upper diagonal pair: chunks (4w+2, 4w+3)
                # only feed queries >= 256 (their tri-masked PV extents)
                e0 = 0
                if (w, h) != (0, 0) and 2 * g2 == 4 * w + 2:
                    e0 = 256
                ex = expool.tile([P, 2, 512], f16, tag="ex")
                nc.scalar.activation(ex[:, :, e0:], sc[:, :, e0:], Exp,
                                     scale=0.25, bias=eshift_sb[:])
                act_ns = (1024 - 2 * e0) * 0.853 + ACT_OH
                # background fill BEFORE the PV matmuls: PV waits on exp, and
                # anything emitted after it would head-of-line block the PE
                fill(act_ns, pe_ns)
                for rr in range(2):
                    kc = 2 * g2 + rr
                    r = kc - 4 * w            # >=0 only inside the diag block
                    q0 = 0
                    if r >= 0:
                        # only q >= r*P can attend this chunk; clip the PV
                        # extent instead of zero-filling the masked region
                        q0 = r * P
                        nc.vector.tensor_mul(ex[:, rr, q0:q0 + P],
                                             ex[:, rr, q0:q0 + P], tri[:])
                    nc.tensor.matmul(pv[:, q0:], Vt[:, kc, h * 65:(h + 1) * 65],
                                     ex[:, rr, q0:],
                                     start=(kc == 0), stop=(kc == last_kc))
            # stash unnormalized O scaled by 1/4 (fp16 range headroom);
            # scatter the denominator into row h%4 of this half's den psum
            nc.vector.tensor_scalar_mul(ot[po:po + 64, ct, :], pv[0:64, :], 0.25)
            den_h = denpool.tile([1, 512], f16, tag="den")
            nc.vector.tensor_copy(den_h, pv[64:65, :])
            # cover the DVE copy latency before the PE scatter consumes it
            fill(700.0, 0.0)
            den_ps = den_ps_ab[h // 4]
            hh = h % 4
            oh = onehot[:, h, 0:4] if h < 4 else onehot[:, h, 4:8]
            nc.tensor.matmul(den_ps, oh, den_h[:],
                             start=(hh == 0), stop=(hh == 3))
            fill(0.0, 512 * 0.4166)

        def norm_units(w, half, ot, den_ps, bc_pools=None):
            """Softmax normalization of ot columns 2*half..2*half+1 (heads
            4*half..4*half+3), reading this half's denominator psum."""
            bc_pools = bc_pools or [pMISC]
            shared = {}

            def recip_unit(ot=ot, den_ps=den_ps):
                rall = rhpool.tile([4, 512], f32, tag="rall")
                nc.vector.reciprocal_approx_fast(rall, den_ps[:])
                rh = rhpool.tile([4, 512], f16, tag="rh")
                # x4 so small reciprocals clear the fp16 subnormal floor; the
                # 4*(1/4) pair cancels in the final normalize multiply
                nc.vector.tensor_scalar_mul(rh, rall, 4.0)
                shared["rh"] = rh
            yield recip_unit
            for jj in range(2):
                j = 2 * half + jj
                pool = bc_pools[jj % len(bc_pools)]
                def unit(j=j, jj=jj, ot=ot, w=w, pool=pool):
                    rh = shared["rh"]
                    # sel_ab[:, 128jj..] row m selects head 4*half + 2jj+(m>=64)
                    bcp = pool.tile([P, 512], f32, name="bc_ps",
                                    tag="aux" if pool is pAUX
                                    else ("pv" if pool is pPV else "misc"))
                    nc.tensor.matmul(bcp, sel_ab[:, jj * P:(jj + 1) * P],
                                     rh[:], start=True, stop=True)
                    # multiply reads the broadcast factors straight from PSUM
                    nc.vector.tensor_mul(ot[:, j, :], ot[:, j, :], bcp[:])
                yield unit

        # ---------------------------- schedule ----------------------------

        qt_cur = qtw.tile([P, 4, 512], f16, tag="qt")
        xT_cur = xtp.tile([P, DCH, 512], f16, tag="xT", name="xT")
        # startup: x tiles split across both hwdge queues, then wq on the
        # scalar queue and wk on sync in parallel; transpose units rotate
        # through the idle attention psum pools and riffle with the first
        # projection granules so the PE is dense from ~2us on
        xts0 = [xload.tile([P, D], f16, tag="xt", name="xt") for _ in range(4)]
        nc.scalar.dma_start(xts0[0], x_d[0:P, :])
        nc.sync.dma_start(xts0[1], x_d[P:2 * P, :])
        nc.scalar.dma_start(xts0[2], x_d[2 * P:3 * P, :])
        nc.sync.dma_start(xts0[3], x_d[3 * P:4 * P, :])
        for d in range(DCH):
            nc.scalar.dma_start(wq_sb[:, d, :], wq_d[d * P:(d + 1) * P, :])
            nc.sync.dma_start(wk_sb[:, d, :], wk_d[d * P:(d + 1) * P, :])
        for d in range(DCH):
            nc.scalar.dma_start(wv_sb[:, d, :], wv_d[d * P:(d + 1) * P, :])
        for j in range(4):            # proj weights are not needed until w3
            nc.sync.dma_start(wp_sb[:, j, :], wp_d[j * P:(j + 1) * P, :])

        tr0 = list(xT_granules(0, xts0, xT_cur, pools=[pPV, pDEN]))
        qk0 = list(qk_granules(0, xT_cur, qt_cur, pools=[pAUX]))
        v0 = list(v_granules(0, xT_cur, pools=[pMISC]))
        tr0[0]()
        tr0[1]()
        qk0[0]()
        tr0[2]()
        qk0[1]()
        tr0[3]()
        for u in qk0[2:]:
            u()
        for u in v0:
            u()

        pending = []            # norm units that must run after window w
        deferred_proj = []      # proj granules, all pushed into window 3
        for w in range(NW):
            ot_cur = otw.tile([P, 4, 512], f16, tag="ot")
            den_a = pDEN.tile([4, 512], f32, tag="den_ps", name="den_a")
            den_b = pDEN.tile([4, 512], f32, tag="den_ps", name="den_b")

            bg = list(pending)
            pending = []
            if w == NW - 1:
                bg += deferred_proj
                deferred_proj = []
            if w + 1 < NW:
                qt_next = qtw.tile([P, 4, 512], f16, tag="qt")
                xT_next = xtp.tile([P, DCH, 512], f16, tag="xT", name="xT")
                xts_n = [xload.tile([P, D], f16, tag="xt", name="xt")
                         for _ in range(4)]
                x_dma(w + 1, xts_n, eng=nc.sync)
                bg += list(xT_granules(w + 1, xts_n, xT_next, pools=[pAUX]))
                bg += list(qk_granules(w + 1, xT_next, qt_next))
                bg += list(v_granules(w + 1, xT_next, pools=[pAUX]))
            else:
                qt_next = xT_next = None
            state["bg"] = bg
            state["debt"] = 0.0

            norm_a = list(norm_units(w, 0, ot_cur, den_a))
            for h in range(NH):
                attention_head(w, h, qt_cur, ot_cur, (den_a, den_b))
                if h == 3:
                    # run the half-a reciprocal now (DVE-only, frees den_a
                    # before h4's scatter rotates the den psum); the bc+mul
                    # units become background work for the second half
                    norm_a[0]()
                    state["bg"] = norm_a[1:] + state["bg"]
            for u in state["bg"]:
                u()
            state["bg"] = []

            norm_b = list(norm_units(w, 1, ot_cur, den_b,
                                     bc_pools=[pMISC, pAUX]
                                     if w == NW - 1 else None))
            if w + 1 < NW:
                norm_b[0]()          # reciprocal frees den_b psum early
                pending = norm_b[1:]
                # deferred projections run inside window 3, where pAUX is
                # otherwise idle (norm/bc stay on pMISC, so the norm-half-a
                # units can be inserted mid-unit without psum aliasing)
                deferred_proj += list(proj_granules(w, ot_cur, pools=[pAUX]))
            else:
                for u in norm_b:
                    u()
                for u in proj_granules(w, ot_cur,
                                       pools=[pMISC, pAUX, pPV]):
                    u()

            qt_cur, xT_cur = qt_next, xT_next

    nc.compile()
    _cache[key] = nc
    return nc


def _make_in_maps(input_data, w_qkv, b_qkv, w_proj, b_proj):
    x = np.asarray(input_data, dtype=np.float32).astype(np.float16)
    wqkv = np.asarray(w_qkv, dtype=np.float32).astype(np.float16)
    bqkv = np.asarray(b_qkv, dtype=np.float32)
    wp = np.asarray(w_proj, dtype=np.float32).astype(np.float16)
    bp = np.asarray(b_proj, dtype=np.float32)

    in_maps = []
    for core in range(NCORES):
        b, g = core // 2, core % 2
        cs = slice(g * C, (g + 1) * C)
        bq = bqkv[0 * D:1 * D][cs]
        bk = bqkv[1 * D:2 * D][cs]
        bqk = np.empty((P, 8), np.float32)
        for j in range(4):
            bqk[:, j] = bq[j * P:(j + 1) * P]
            bqk[:, 4 + j] = bk[j * P:(j + 1) * P]
        in_maps.append({
            "x": np.ascontiguousarray(x[b]),
            "wq": np.ascontiguousarray(wqkv[:, 0 * D:1 * D][:, cs]),
            "wk": np.ascontiguousarray(wqkv[:, 1 * D:2 * D][:, cs]),
            "wv": np.ascontiguousarray(wqkv[:, 2 * D:3 * D][:, cs]),
            "wp": np.ascontiguousarray(wp[cs, :]),
            "bqk": bqk,
            "bv": np.ascontiguousarray(bqkv[2 * D:3 * D][cs]).reshape(1, C),
            "bp": (bp if g == 0 else np.zeros_like(bp)).reshape(1, D),
        })
    return in_maps


def kernel(input_data, w_qkv, b_qkv, w_proj, b_proj):
    from concourse.bass_utils import run_bass_kernel_spmd

    has_bias = bool(np.any(np.asarray(b_qkv)) or np.any(np.asarray(b_proj)))
    nc = _build(has_bias)
    in_maps = _make_in_maps(input_data, w_qkv, b_qkv, w_proj, b_proj)
    res = run_bass_kernel_spmd(nc, in_maps, core_ids=list(range(NCORES)))
    parts = [np.asarray(res.results[i]["out"], dtype=np.float32)
             for i in range(NCORES)]
    out = np.stack([parts[2 * b] + parts[2 * b + 1] for b in range(B)])
    return out.astype(np.float32)


# revision 30
# speedup vs baseline: 1.4309x; 1.0769x over previous
"""Causal self-attention (B=4, S=2048, D=1024, H=16) on 8 TRN2 NeuronCores.

Sharding: core = (batch b, head-group g) with b = core//2, g = core%2.
Each core computes, for its batch and its 8 heads: QKV projection,
transposed flash-style attention (scores kept k-major so the softmax'd
weights feed the PV matmul directly as the moving operand), and a partial
output projection over its heads' 512 hidden dims.  The host sums the two
partial projections per batch.

Perf notes (v4):
- x is transposed on the PE in d-major units (all four seq-tiles of one
  dh-pair per unit), so the first Q-projection granule's moving operand is
  complete after one unit (~0.5us of PE).  DMA-crossbar transposes were
  tried and are ~10x slower than the cost model claims on this hardware.
- Startup streams x on both hwdge queues and weights on the scalar queue
  in parallel; transpose/projection units rotate across the idle attention
  psum pools so nothing serializes on a single psum bank.
- Background PE work is emitted in ~2-matmul granules BETWEEN a group's
  score matmuls and its PV matmuls (the PV waits on exp; granules emitted
  after it would head-of-line block the in-order PE queue).  TRN2 drops the
  PE clock from 2.4 to 1.2 GHz for ~3us after every idle gap, so the PE
  must never go idle.
- exp() extents are clipped on the causal-diagonal chunk pairs.
- The softmax denominator never touches a psum scatter: each head's ones-row
  is copied (DVE for even rows, small SBUF-to-SBUF DMA for odd rows - DMA
  writes have no 32-partition alignment rule) into a per-head-pair [2,512]
  SBUF tile, and each pair is normalized as soon as both heads finish
  (heads run odd-first within a pair so no DMA sits on the reciprocal path).
- All output projections are deferred into window 3, which is otherwise
  exp(ACT)-bound.
- Output is stored fp16 (host accumulates partials in fp32).

Numerics: matmul operands are fp16 (1 cycle/row on the PE vs 2 for fp32r,
accumulation still fp32 in PSUM); exp runs with a constant -4 shift, tuned so
fp16 softmax weights neither overflow nor hit the subnormal flush-to-zero
floor (the shift cancels in normalization).
End-to-end relative error ~1e-3 vs the fp32 reference.
"""

import numpy as np

B, S, D, H = 4, 2048, 1024, 16
HS = 64            # head size
NH = 8             # heads per core
C = 512            # per-core q/k/v width (NH * HS)
P = 128
NCORES = 8
DCH = D // P       # 8 contraction chunks for the projections
NW = S // 512      # 4 query windows of 512
KCH = S // P       # 16 key chunks
ESHIFT = -4.0      # exp(0.25*s + ESHIFT); cancels in the softmax ratio

_cache = {}


def _build(has_bias=False):
    key = ("nc", has_bias)
    if key in _cache:
        return _cache[key]

    from contextlib import ExitStack

    import concourse.bass as bass
    import concourse.tile as tile
    from concourse import bacc, mybir
    from concourse.masks import make_identity, make_upper_triangular

    f32 = mybir.dt.float32
    f16 = mybir.dt.float16
    Exp = mybir.ActivationFunctionType.Exp

    nc = bacc.Bacc(
        "TRN2", target_bir_lowering=False, debug=False, num_devices=NCORES
    )

    x_d = nc.dram_tensor("x", [S, D], f16, kind="ExternalInput").ap()
    wq_d = nc.dram_tensor("wq", [D, C], f16, kind="ExternalInput").ap()
    wk_d = nc.dram_tensor("wk", [D, C], f16, kind="ExternalInput").ap()
    wv_d = nc.dram_tensor("wv", [D, C], f16, kind="ExternalInput").ap()
    wp_d = nc.dram_tensor("wp", [C, D], f16, kind="ExternalInput").ap()
    # per-partition bias columns: col j<4 -> q col-tile j, col 4+j -> k col-tile j
    bqk_d = nc.dram_tensor("bqk", [P, 8], f32, kind="ExternalInput").ap()
    bv_d = nc.dram_tensor("bv", [1, C], f32, kind="ExternalInput").ap()
    bp_d = nc.dram_tensor("bp", [1, D], f32, kind="ExternalInput").ap()
    out_d = nc.dram_tensor("out", [S, D], f16, kind="ExternalOutput").ap()

    with tile.TileContext(nc) as tc, ExitStack() as ctx:
        ctx.enter_context(nc.allow_low_precision(reason="fp16 attention"))

        const = ctx.enter_context(tc.tile_pool(name="const", bufs=1))
        persist = ctx.enter_context(tc.tile_pool(name="persist", bufs=1))
        xload = ctx.enter_context(tc.tile_pool(name="xload", bufs=8))
        xtp = ctx.enter_context(tc.tile_pool(name="xtp", bufs=2))
        qtw = ctx.enter_context(tc.tile_pool(name="qtw", bufs=2))
        otw = ctx.enter_context(tc.tile_pool(name="otw", bufs=4))
        expool = ctx.enter_context(tc.tile_pool(name="expool", bufs=4))
        denpool = ctx.enter_context(tc.tile_pool(name="denpool", bufs=3))
        dpairs = ctx.enter_context(tc.tile_pool(name="dpairs", bufs=8))
        rhpool = ctx.enter_context(tc.tile_pool(name="rhpool", bufs=4))
        stpool = ctx.enter_context(tc.tile_pool(name="stpool", bufs=3))

        pS = ctx.enter_context(tc.tile_pool(name="pS", bufs=2, space="PSUM"))
        pPV = ctx.enter_context(tc.tile_pool(name="pPV", bufs=1, space="PSUM"))
        pMISC = ctx.enter_context(tc.tile_pool(name="pMISC", bufs=1, space="PSUM"))
        pAUX = ctx.enter_context(tc.tile_pool(name="pAUX", bufs=2, space="PSUM"))

        identf = const.tile([P, P], f32, tag="identf")
        make_identity(nc, identf)
        ident = const.tile([P, P], f16, tag="ident")
        nc.vector.tensor_copy(ident, identf)
        tri = const.tile([P, P], f32, tag="tri")
        make_upper_triangular(nc, tri, val=1.0, diag=True)  # tri[k,q]=1 iff q>=k
        eshift_sb = const.tile([P, 1], f32, tag="eshift")
        nc.vector.memset(eshift_sb, ESHIFT)
        ones8 = const.tile([P, 8], f32, tag="ones8")
        nc.vector.memset(ones8, 1.0)
        # head-pair band select: within any 128-col tile j, the first 64 cols
        # belong to head 2j (rh row 0), the last 64 to head 2j+1 (row 1)
        bandf = const.tile([2, 128], f32, tag="bandf")
        nc.gpsimd.memset(bandf, 1.0)
        nc.gpsimd.affine_select(
            out=bandf, in_=bandf, compare_op=mybir.AluOpType.is_ge,
            fill=0.0, base=0, pattern=[[1, 128]], channel_multiplier=-64)
        nc.gpsimd.affine_select(
            out=bandf, in_=bandf, compare_op=mybir.AluOpType.is_ge,
            fill=0.0, base=63, pattern=[[-1, 128]], channel_multiplier=64)
        sel_p = const.tile([2, 128], f16, tag="sel_p")
        nc.vector.tensor_copy(sel_p, bandf[:])

        if has_bias:
            bqk_sb = const.tile([P, 8], f32, tag="bqk")
            nc.sync.dma_start(bqk_sb, bqk_d)
            bv_bc = const.tile([P, C], f32, tag="bv_bc")
            nc.sync.dma_start(
                bv_bc,
                bass.AP(tensor=bv_d.tensor, offset=bv_d.offset,
                        ap=[[0, P], list(bv_d.ap[-1])]),
            )
            bp_bc = const.tile([P, D], f32, tag="bp_bc")
            nc.sync.dma_start(
                bp_bc,
                bass.AP(tensor=bp_d.tensor, offset=bp_d.offset,
                        ap=[[0, P], list(bp_d.ap[-1])]),
            )

        wq_sb = persist.tile([P, DCH, C], f16, tag="wq")
        wk_sb = persist.tile([P, DCH, C], f16, tag="wk")
        wv_sb = persist.tile([P, DCH, C], f16, tag="wv")
        wp_sb = persist.tile([P, 4, D], f16, tag="wp")
        KT = persist.tile([P, 4, S], f16, tag="KT")
        Vt = persist.tile([P, KCH, NH * 65], f16, tag="Vt")

        # ---------- unit generators (emitted lazily for interleaving) ----------

        def x_dma(w, xts, eng=None):
            """Plain DMA of window w's x rows into 4 seq-tiles [P, D]."""
            eng = eng or nc.sync
            for st in range(4):
                row0 = (4 * w + st) * P
                eng.dma_start(xts[st], x_d[row0:row0 + P, :])

        def xT_granules(w, xts, xT, pools=None):
            """PE-transpose window w's x into xT [P, DCH, 512], one dh-PAIR
            per granule (all 4 seq-tiles), so consumers needing low dh chunks
            unblock after a single granule.  psum slot order (dhh, st)."""
            pools = pools or [pAUX]
            for jd in range(4):
                pool = pools[jd % len(pools)]
                def unit(w=w, jd=jd, xts=xts, xT=xT, pool=pool):
                    ptr = pool.tile([P, 2, 4, P], f16, name="tr_ps",
                                    tag="aux" if pool is pAUX
                                    else ("pv" if pool is pPV else "misc"))
                    for dhh in range(2):
                        d = 2 * jd + dhh
                        for st in range(4):
                            nc.tensor.transpose(
                                ptr[:, dhh, st, :],
                                xts[st][:, d * P:(d + 1) * P], ident[:])
                    nc.vector.tensor_copy(
                        xT[:, 2 * jd:2 * jd + 2, :]
                        .rearrange("p a (b c) -> p a b c", c=P), ptr[:])
                yield unit

        def qk_granules(w, xT, qt, pools=None, order=None):
            """Q^T / K^T projections for window w from xT, in 2-matmul granules.
            Order: ct-major with q before k so head 2ct/2ct+1's inputs complete
            as early as possible."""
            pools = pools or [pAUX]
            units = order or [(ct, qk) for ct in range(4) for qk in range(2)]
            for ui, (ct, qk) in enumerate(units):
                ps = [None]
                pool = pools[ui % len(pools)]
                for gd in range(4):
                    def unit(w=w, ct=ct, qk=qk, gd=gd, xT=xT, qt=qt, ps=ps,
                             pool=pool):
                        if gd == 0:
                            ps[0] = pool.tile([P, 512], f32, name="aux_ps",
                                              tag="aux" if pool is pAUX
                                              else "misc")
                        wsb = wq_sb if qk == 0 else wk_sb
                        for d in (2 * gd, 2 * gd + 1):
                            nc.tensor.matmul(
                                ps[0], wsb[:, d, ct * P:(ct + 1) * P],
                                xT[:, d, :],
                                start=(d == 0), stop=(d == DCH - 1))
                        if gd == 3:
                            dest = (qt[:, ct, :] if qk == 0
                                    else KT[:, ct, w * 512:(w + 1) * 512])
                            if has_bias:
                                nc.vector.tensor_scalar_add(
                                    dest, ps[0],
                                    bqk_sb[:, qk * 4 + ct:qk * 4 + ct + 1])
                            else:
                                nc.vector.tensor_copy(dest, ps[0])
                    yield unit

        def v_granules(w, xT, pools=None):
            """V for the 4 key chunks of window w, head-grouped with ones col."""
            pools = pools or [pAUX]
            for st in range(4):
                ps = [None]
                pool = pools[st % len(pools)]
                for gd in range(4):
                    def unit(w=w, st=st, gd=gd, xT=xT, ps=ps, pool=pool):
                        if gd == 0:
                            ps[0] = pool.tile([P, 512], f32, name="aux_ps",
                                              tag="aux" if pool is pAUX
                                              else "misc")
                        for d in (2 * gd, 2 * gd + 1):
                            nc.tensor.matmul(ps[0],
                                             xT[:, d, st * P:(st + 1) * P],
                                             wv_sb[:, d, :],
                                             start=(d == 0), stop=(d == DCH - 1))
                        if gd == 3:
                            kc = 4 * w + st
                            vtv = Vt[:, kc, :].rearrange("p (h c) -> p h c", c=65)
                            if has_bias:
                                nc.vector.tensor_add(
                                    vtv[:, :, 0:64],
                                    ps[0].rearrange("p (h c) -> p h c", c=64),
                                    bv_bc[:].rearrange("p (h c) -> p h c", c=64))
                            else:
                                nc.vector.tensor_copy(
                                    vtv[:, :, 0:64],
                                    ps[0].rearrange("p (h c) -> p h c", c=64))
                            nc.vector.tensor_copy(vtv[:, :, 64:65],
                                                  ones8[:, :].unsqueeze(2))
                    yield unit

        def proj_granules(w, ot, pools=None):
            """Output projection for window w's 512 seq rows (partial over C),
            2-matmul granules; the closing granule stages and DMAs out fp16."""
            pools = pools or [pMISC]
            for ui in range(8):
                st, cw = ui // 2, ui % 2
                pp = [None]
                pool = pools[ui % len(pools)]
                for gj in range(2):
                    def unit(w=w, st=st, cw=cw, gj=gj, ot=ot, pp=pp, pool=pool):
                        if gj == 0:
                            pp[0] = pool.tile([P, 512], f32, name="proj_pp",
                                              tag="aux" if pool is pAUX
                                              else ("pv" if pool is pPV
                                                    else "misc"))
                        for j in (2 * gj, 2 * gj + 1):
                            nc.tensor.matmul(
                                pp[0], ot[:, j, st * P:(st + 1) * P],
                                wp_sb[:, j, cw * 512:(cw + 1) * 512],
                                start=(j == 0), stop=(j == 3))
                        if gj == 1:
                            stg = stpool.tile([P, 512], f16, tag="stg")
                            if has_bias:
                                nc.vector.tensor_add(
                                    stg, pp[0],
                                    bp_bc[:, cw * 512:(cw + 1) * 512])
                            else:
                                nc.vector.tensor_copy(stg, pp[0])
                            row0 = (4 * w + st) * P
                            nc.sync.dma_start(
                                out_d[row0:row0 + P,
                                      cw * 512:(cw + 1) * 512], stg)
                    yield unit

        # debt-based background filler -------------------------------------
        PE_NS = 0.55             # measured ns per matmul row (sustained)
        GRAN_NS = 600.0          # ~2 fp16 matmuls of 512 rows, measured
        ACT_OH = 262.0           # measured ACT fixed overhead per instruction
        state = {"debt": 0.0, "bg": []}

        def fill(act_ns, pe_ns):
            """Inject background granules to cover PE idle while the scalar
            engine runs exp (or a cross-engine dependency settles)."""
            state["debt"] += act_ns - pe_ns
            while state["debt"] > 0.5 * GRAN_NS and state["bg"]:
                state["bg"].pop(0)()
                state["debt"] -= GRAN_NS
            if state["debt"] < -3 * GRAN_NS:
                state["debt"] = -3 * GRAN_NS

        first_head = [True]

        def attention_head(w, h, qt, ot, den_pair):
            """Scores+exp+PV for one head; unnormalized O -> ot, denominator
            row -> den_pair[h%2] (DVE when row 0, small DMA when row 1)."""
            ct, po = h // 2, (h % 2) * 64
            pv = pPV.tile([65, 512], f32, tag="pv")
            last_kc = 4 * w + 3
            first = first_head[0]
            first_head[0] = False
            for g2 in range(2 * (w + 1)):      # 2-chunk half groups
                sc = pS.tile([P, 2, 512], f32, tag="sc")
                pe_ns = 0.0
                for rr in range(2):
                    kc = 2 * g2 + rr
                    # masked q-range is never read downstream; keep the very
                    # first head full-extent so the psum slots are initialized
                    s0 = 0 if first else max(0, kc - 4 * w) * P
                    nc.tensor.matmul(
                        sc[:, rr, s0:],
                        KT[po:po + 64, ct, kc * P:(kc + 1) * P],
                        qt[po:po + 64, ct, s0:],
                        start=True, stop=True)
                    pe_ns += (512 - s0) * PE_NS
                # clip exp on the upper diagonal pair: chunks (4w+2, 4w+3)
                # only feed queries >= 256 (their tri-masked PV extents)
                e0 = 0
                if not first and 2 * g2 == 4 * w + 2:
                    e0 = 256
                ex = expool.tile([P, 2, 512], f16, tag="ex")
                nc.scalar.activation(ex[:, :, e0:], sc[:, :, e0:], Exp,
                                     scale=0.25, bias=eshift_sb[:])
                act_ns = (1024 - 2 * e0) * 0.853 + ACT_OH
                # background fill BEFORE the PV matmuls: PV waits on exp, and
                # anything emitted after it would head-of-line block the PE
                fill(act_ns, pe_ns)
                for rr in range(2):
                    kc = 2 * g2 + rr
                    r = kc - 4 * w            # >=0 only inside the diag block
                    q0 = 0
                    if r >= 0:
                        # only q >= r*P can attend this chunk; clip the PV
                        # extent instead of zero-filling the masked region
                        q0 = r * P
                        nc.vector.tensor_mul(ex[:, rr, q0:q0 + P],
                                             ex[:, rr, q0:q0 + P], tri[:])
                    nc.tensor.matmul(pv[:, q0:], Vt[:, kc, h * 65:(h + 1) * 65],
                                     ex[:, rr, q0:],
                                     start=(kc == 0), stop=(kc == last_kc))
            # stash unnormalized O scaled by 1/4 (fp16 range headroom)
            nc.vector.tensor_scalar_mul(ot[po:po + 64, ct, :], pv[0:64, :], 0.25)
            if h % 2 == 0:
                # even head -> pair row 0: direct DVE copy (aligned)
                nc.vector.tensor_copy(den_pair[0:1, :], pv[64:65, :])
            else:
                # odd head -> pair row 1: partition 1 is unaligned for the
                # DVE, so stage on partition 0 and let a 2KB DMA place it
                den_h = denpool.tile([1, 512], f32, tag="den")
                nc.vector.tensor_copy(den_h, pv[64:65, :])
                nc.sync.dma_start(den_pair[1:2, :], den_h)

        def pair_norm_units(j, ot, den_pair, bc_pool=None):
            """Normalize ot column-tile j (heads 2j, 2j+1) from den_pair."""
            bc_pool = bc_pool or pMISC
            shared = {}

            def recip_unit(den_pair=den_pair):
                rall = rhpool.tile([2, 512], f32, tag="rall")
                nc.vector.reciprocal_approx_fast(rall, den_pair[:])
                rh = rhpool.tile([2, 512], f16, tag="rh")
                # x4 so small reciprocals clear the fp16 subnormal floor; the
                # 4*(1/4) pair cancels in the final normalize multiply
                nc.vector.tensor_scalar_mul(rh, rall, 4.0)
                shared["rh"] = rh
            yield recip_unit

            def bcmul_unit(j=j, ot=ot, pool=bc_pool):
                rh = shared["rh"]
                bcp = pool.tile([P, 512], f32, name="bc_ps",
                                tag="aux" if pool is pAUX
                                else ("pv" if pool is pPV else "misc"))
                nc.tensor.matmul(bcp, sel_p[:, :], rh[:], start=True, stop=True)
                # multiply reads the broadcast factors straight from PSUM
                nc.vector.tensor_mul(ot[:, j, :], ot[:, j, :], bcp[:])
            yield bcmul_unit

        # ---------------------------- schedule ----------------------------

        qt_cur = qtw.tile([P, 4, 512], f16, tag="qt")
        xT_cur = xtp.tile([P, DCH, 512], f16, tag="xT", name="xT")
        # startup: x tiles split across both hwdge queues, then wq on the
        # scalar queue and wk on sync in parallel; transpose units rotate
        # through the idle attention psum pools and riffle with the first
        # projection granules so the PE is dense from ~2us on
        xts0 = [xload.tile([P, D], f16, tag="xt", name="xt") for _ in range(4)]
        nc.scalar.dma_start(xts0[0], x_d[0:P, :])
        nc.sync.dma_start(xts0[1], x_d[P:2 * P, :])
        nc.scalar.dma_start(xts0[2], x_d[2 * P:3 * P, :])
        nc.sync.dma_start(xts0[3], x_d[3 * P:4 * P, :])
        for d in range(DCH):
            nc.scalar.dma_start(wq_sb[:, d, :], wq_d[d * P:(d + 1) * P, :])
            nc.sync.dma_start(wk_sb[:, d, :], wk_d[d * P:(d + 1) * P, :])
        for d in range(DCH):
            nc.scalar.dma_start(wv_sb[:, d, :], wv_d[d * P:(d + 1) * P, :])
        for j in range(4):            # proj weights are not needed until w3
            nc.sync.dma_start(wp_sb[:, j, :], wp_d[j * P:(j + 1) * P, :])

        tr0 = list(xT_granules(0, xts0, xT_cur, pools=[pPV, pMISC]))
        qk0 = list(qk_granules(0, xT_cur, qt_cur, pools=[pAUX]))
        v0 = list(v_granules(0, xT_cur, pools=[pMISC]))
        tr0[0]()
        tr0[1]()
        qk0[0]()
        tr0[2]()
        qk0[1]()
        tr0[3]()
        for u in qk0[2:]:
            u()
        for u in v0:
            u()

        deferred_proj = []      # proj granules, all pushed into window 3
        # odd head of each pair first, so the odd row's den DMA is in flight
        # while the even head computes and no DMA sits on the recip path
        HEAD_ORDER = [1, 0, 3, 2, 5, 4, 7, 6]
        last_norm = []
        for w in range(NW):
            ot_cur = otw.tile([P, 4, 512], f16, tag="ot")

            bg = []
            if w == NW - 1:
                bg += deferred_proj
                deferred_proj = []
            if w + 1 < NW:
                qt_next = qtw.tile([P, 4, 512], f16, tag="qt")
                xT_next = xtp.tile([P, DCH, 512], f16, tag="xT", name="xT")
                xts_n = [xload.tile([P, D], f16, tag="xt", name="xt")
                         for _ in range(4)]
                x_dma(w + 1, xts_n, eng=nc.sync)
                bg += list(xT_granules(w + 1, xts_n, xT_next, pools=[pAUX]))
                bg += list(qk_granules(w + 1, xT_next, qt_next))
                bg += list(v_granules(w + 1, xT_next, pools=[pAUX]))
            else:
                qt_next = xT_next = None
            state["bg"] = bg
            state["debt"] = 0.0

            den_tiles = [dpairs.tile([2, 512], f32, tag="dpair", name="dpair")
                         for _ in range(4)]
            for h in HEAD_ORDER:
                attention_head(w, h, qt_cur, ot_cur, den_tiles[h // 2])
                if h % 2 == 0:      # pair (2j, 2j+1) complete (odd ran first)
                    j = h // 2
                    units = list(pair_norm_units(j, ot_cur, den_tiles[j]))
                    if (w, j) == (NW - 1, 3):
                        last_norm = units       # tail: run right after flush
                    else:
                        state["bg"] = units + state["bg"]
            for u in state["bg"]:
                u()
            state["bg"] = []

            if w + 1 < NW:
                # deferred projections run inside window 3, where pAUX is
                # otherwise idle (norm/bc stay on pMISC, so pair-norm units
                # can be inserted mid-stream without psum aliasing)
                deferred_proj += list(proj_granules(w, ot_cur, pools=[pAUX]))
            else:
                for u in last_norm:
                    u()
                for u in proj_granules(w, ot_cur, pools=[pMISC, pAUX]):
                    u()

            qt_cur, xT_cur = qt_next, xT_next

    nc.compile()
    _cache[key] = nc
    return nc


def _make_in_maps(input_data, w_qkv, b_qkv, w_proj, b_proj):
    x = np.asarray(input_data, dtype=np.float32).astype(np.float16)
    wqkv = np.asarray(w_qkv, dtype=np.float32).astype(np.float16)
    bqkv = np.asarray(b_qkv, dtype=np.float32)
    wp = np.asarray(w_proj, dtype=np.float32).astype(np.float16)
    bp = np.asarray(b_proj, dtype=np.float32)

    in_maps = []
    for core in range(NCORES):
        b, g = core // 2, core % 2
        cs = slice(g * C, (g + 1) * C)
        bq = bqkv[0 * D:1 * D][cs]
        bk = bqkv[1 * D:2 * D][cs]
        bqk = np.empty((P, 8), np.float32)
        for j in range(4):
            bqk[:, j] = bq[j * P:(j + 1) * P]
            bqk[:, 4 + j] = bk[j * P:(j + 1) * P]
        in_maps.append({
            "x": np.ascontiguousarray(x[b]),
            "wq": np.ascontiguousarray(wqkv[:, 0 * D:1 * D][:, cs]),
            "wk": np.ascontiguousarray(wqkv[:, 1 * D:2 * D][:, cs]),
            "wv": np.ascontiguousarray(wqkv[:, 2 * D:3 * D][:, cs]),
            "wp": np.ascontiguousarray(wp[cs, :]),
            "bqk": bqk,
            "bv": np.ascontiguousarray(bqkv[2 * D:3 * D][cs]).reshape(1, C),
            "bp": (bp if g == 0 else np.zeros_like(bp)).reshape(1, D),
        })
    return in_maps


def kernel(input_data, w_qkv, b_qkv, w_proj, b_proj):
    from concourse.bass_utils import run_bass_kernel_spmd

    has_bias = bool(np.any(np.asarray(b_qkv)) or np.any(np.asarray(b_proj)))
    nc = _build(has_bias)
    in_maps = _make_in_maps(input_data, w_qkv, b_qkv, w_proj, b_proj)
    res = run_bass_kernel_spmd(nc, in_maps, core_ids=list(range(NCORES)))
    parts = [np.asarray(res.results[i]["out"], dtype=np.float32)
             for i in range(NCORES)]
    out = np.stack([parts[2 * b] + parts[2 * b + 1] for b in range(B)])
    return out.astype(np.float32)
